# revision 31
# baseline (speedup 1.0000x reference)
"""Trainium2 Bass kernel for nn_BidirectionalAttention (B=16,H=4,T=256,N=2048,D=256).

Math (reference):
    Qr = rope2d(Q), Kr = rope2d(K)              # elementwise, per (t, n) angle
    scores = Qr @ Kr^T / sqrt(N)                # (B,H,T,T), no softmax
    out    = scores @ V                         # V (B,1,T,D) broadcasts over H

Strategy (B sharded 2-per-core across 8 cores):
  * Host-side input prep: features are permuted to evens-first order
    (n' = [0,2,..,2046,1,3,..,2047]); the contraction over n is
    permutation-invariant so scores are unchanged.  Under this order the
    RoPE pair-swap becomes a +-1024 BLOCK swap, which in the on-chip
    chunked layout is pure tile indexing (chunk j <-> j^8).
  * Q and K are uploaded bf16 PRE-TRANSPOSED as [n, t] chunk tiles
    ([128, NCH*T] per (b,h)), so the kernel needs NO on-device
    transposes at all (the baseline spent most of its PE time on 128
    transpose matmuls per (b,h)).  bf16 upload also halves HBM traffic.
  * RoPE in transposed space:  QrT_j = cT_j*QT_j + sT_j*QT_{j^8}.
    K side is materialized with 2 muls + 1 add (DVE + GPSIMD).
    Q side's add is FOLDED into mm1's PSUM accumulation:
        scoresT = sum_j  KrT_j^T @ A_j  +  KrT_j^T @ B_{j^8}
    with A = cT.QT, B = sTswap.QT  (2 muls, no add); each lhsT serves
    two consecutive matmuls (one weight load per chunk).
  * mm2: out[t,d] = sum_s scoresT[s,t] @ V[s,d], V uploaded bf16 in
    [s, d] chunk layout (no transpose needed anywhere).
  * 1/sqrt(N) folded into the tables (N^-1/4 on both Q and K sides).
"""

import math
import os
import numpy as np
import ml_dtypes
from contextlib import ExitStack

import concourse.bass as bass
import concourse.bacc as bacc_mod
import concourse.tile as tile
import concourse.mybir as mybir
from concourse.bass_utils import run_bass_kernel_spmd
from concourse.masks import make_identity

bf16 = ml_dtypes.bfloat16

# problem shapes (hardcoded per contract)
B, H, T, N, D = 16, 4, 256, 2048, 256
GRID = 16
THETA = 10000.0
NCORES = 8
BS = B // NCORES          # batches per core
P = 128
NCH = N // P              # 16 feature chunks
TCH = T // P              # 2 token chunks
HALF = N // 2

LAST_RESULT = None        # BassKernelResults of the most recent run (for test.py)


def _tile_pack(x):
    """[T, N]-indexed table -> SBUF tile layout [128, NCH*T]:
    tile[p, j*T + t] = x[t, j*128 + p]."""
    return np.ascontiguousarray(
        x.T.reshape(NCH, P, T).transpose(1, 0, 2).reshape(P, NCH * T))


def _rope_tables():
    """Tables in permuted (evens-first) transposed tile layout, bf16.

    ct[p, j*T+t] = alpha * cos(ang[t, perm[j*128+p]])
    st[p, j*T+t] = alpha * stilde[t, j*128+p]  where
       stilde[t, m <1024] = -sin(ang[t, 2m])      (coeff of Q'[m+1024])
       stilde[t, m>=1024] = +sin(ang[t, 2(m-1024)+1])  (coeff of Q'[m-1024])
    """
    inv_freq = (1.0 / THETA ** (np.arange(0, HALF, 2, dtype=np.float32)
                                / np.float32(HALF))).astype(np.float32)
    pos = np.arange(GRID, dtype=np.float32)
    ph = pos[:, None] * inv_freq[None, :]                      # (16, 512)
    ph_h = np.broadcast_to(ph[:, None, :], (GRID, GRID, HALF // 2))
    ph_w = np.broadcast_to(ph[None, :, :], (GRID, GRID, HALF // 2))
    phases = np.concatenate([ph_h, ph_w, ph_h, ph_w], axis=-1).reshape(T, N)
    ang = np.mod(phases, np.float32(1.0)) * np.float32(2.0 * math.pi)
    c = np.cos(ang)
    s = np.sin(ang)
    alpha = np.float32(1.0 / math.sqrt(math.sqrt(N)))
    perm = np.concatenate([np.arange(0, N, 2), np.arange(1, N, 2)])
    cp = c[:, perm] * alpha
    st = np.empty_like(s)
    st[:, :HALF] = -s[:, 0::2]
    st[:, HALF:] = s[:, 1::2]
    st *= alpha
    return _tile_pack(cp).astype(bf16), _tile_pack(st).astype(bf16)


def _prep_inputs(Q, K, V):
    """Full f32 inputs -> per-core upload dicts (bf16, permuted, transposed,
    chunk-tiled). Tables uploaded as unique halves (chunks 0-3, 8-11)."""
    perm = np.concatenate([np.arange(0, N, 2), np.arange(1, N, 2)])
    ct, st = _rope_tables()
    QT4 = 4 * T
    ct = np.ascontiguousarray(
        np.concatenate([ct[:, 0:QT4], ct[:, 2 * QT4:3 * QT4]], axis=1))
    st = np.ascontiguousarray(
        np.concatenate([st[:, 0:QT4], st[:, 2 * QT4:3 * QT4]], axis=1))

    def pack_qk(x):
        xp = x[:, :, :, perm].astype(bf16)                   # [B,H,T,N]
        xt = xp.transpose(0, 1, 3, 2)                        # [B,H,N,T]
        return np.ascontiguousarray(
            xt.reshape(B, H, NCH, P, T).transpose(0, 1, 3, 2, 4)
              .reshape(B, H, P, NCH * T))

    qt = pack_qk(Q)
    kt = pack_qk(K)
    vt = np.ascontiguousarray(
        V[:, 0].astype(bf16).reshape(B, TCH, P, D).transpose(0, 2, 1, 3)
         .reshape(B, P, TCH * D))

    in_maps = []
    for c in range(NCORES):
        sl = slice(c * BS, (c + 1) * BS)
        in_maps.append({"QT": qt[sl], "KT": kt[sl], "VT": vt[sl],
                        "CT": ct, "ST": st})
    return in_maps


def _unpack_out(per_core):
    """[BS,H,128,TCH*D] bf16 per core -> [B,H,T,D] f32."""
    o = np.concatenate([np.asarray(p) for p in per_core], axis=0)
    return np.ascontiguousarray(
        o.astype(np.float32).reshape(B, H, P, TCH, D).transpose(0, 1, 3, 2, 4)
         .reshape(B, H, T, D))


def _build_nc():
    nc = bacc_mod.Bacc("TRN2", target_bir_lowering=False, debug=False)

    qt_dram = nc.dram_tensor("QT", [BS, H, P, NCH * T], mybir.dt.bfloat16,
                             kind="ExternalInput").ap()
    kt_dram = nc.dram_tensor("KT", [BS, H, P, NCH * T], mybir.dt.bfloat16,
                             kind="ExternalInput").ap()
    vt_dram = nc.dram_tensor("VT", [BS, P, TCH * D], mybir.dt.bfloat16,
                             kind="ExternalInput").ap()
    # tables have exact 2-fold chunk redundancy (chunk j == j+4 for
    # j in {0..3, 8..11}); upload unique half, duplicate on-chip
    c_dram = nc.dram_tensor("CT", [P, NCH * T // 2], mybir.dt.bfloat16,
                            kind="ExternalInput").ap()
    s_dram = nc.dram_tensor("ST", [P, NCH * T // 2], mybir.dt.bfloat16,
                            kind="ExternalInput").ap()
    o_dram = nc.dram_tensor("O", [BS, H, P, TCH * D], mybir.dt.bfloat16,
                            kind="ExternalOutput").ap()

    W = NCH * T          # 4096
    HW = W // 2          # 2048 (chunk-block half: chunks 0-7 | 8-15)

    with tile.TileContext(nc) as tc, ExitStack() as ctx:
        const_pool = ctx.enter_context(tc.tile_pool(name="const", bufs=1))
        qk_pool = ctx.enter_context(tc.tile_pool(name="qk", bufs=3))
        v_pool = ctx.enter_context(tc.tile_pool(name="vp", bufs=2))
        work_pool = ctx.enter_context(tc.tile_pool(name="work", bufs=2))
        sc_pool = ctx.enter_context(tc.tile_pool(name="scp", bufs=2))
        out_pool = ctx.enter_context(tc.tile_pool(name="outp", bufs=2))
        ps_sc = ctx.enter_context(tc.tile_pool(name="pssc", bufs=2, space="PSUM"))
        ps_out = ctx.enter_context(tc.tile_pool(name="psout", bufs=2, space="PSUM"))
        ps_add = ctx.enter_context(tc.tile_pool(name="psadd", bufs=1, space="PSUM"))

        # PE-assisted add: kr chunks 0-5 are summed on the tensor engine
        # (identity-matmul accumulate, exact f32) to offload the DVE
        KA = 6 * T           # 1536 columns of kr via PE, rest via DVE
        ident = const_pool.tile([P, P], mybir.dt.bfloat16)
        make_identity(nc, ident[:])
        ct = const_pool.tile([P, W], mybir.dt.bfloat16)
        st = const_pool.tile([P, W], mybir.dt.bfloat16)
        # ALL loads ride the SP ring in priority order (per-ring transfers
        # complete in issue order; spreading across rings just makes the
        # SDMA round-robin delay the critical ones). Outputs ride the
        # Activation ring. st first (t2 needs kt+st), ct after kt0.
        QT4 = 4 * T   # 1024-col quarter (4 chunks)
        nc.sync.dma_start(st[:, 0:QT4], s_dram[:, 0:QT4])
        nc.sync.dma_start(st[:, 2 * QT4:3 * QT4], s_dram[:, QT4:2 * QT4])
        # duplicate to chunks 4-7 / 12-15 on GPSIMD: keeps the DVE FIFO
        # free, and GPSIMD-DVE port contention is harmless in the head
        # (DVE idles on DMA there anyway)
        nc.gpsimd.tensor_copy(st[:, QT4:2 * QT4], st[:, 0:QT4])
        nc.gpsimd.tensor_copy(st[:, 3 * QT4:4 * QT4], st[:, 2 * QT4:3 * QT4])

        first = True
        for b in range(BS):
            v_bf = v_pool.tile([P, TCH * D], mybir.dt.bfloat16, tag="vbf")
            for h in range(H):
                qt = qk_pool.tile([P, W], mybir.dt.bfloat16, tag="qt")
                kt = qk_pool.tile([P, W], mybir.dt.bfloat16, tag="kt")
                nc.sync.dma_start(kt[:], kt_dram[b, h])
                if first:
                    # ct lands after kt0 (needed by t1, the 3rd DVE op)
                    nc.sync.dma_start(ct[:, 0:QT4], c_dram[:, 0:QT4])
                    nc.sync.dma_start(ct[:, 2 * QT4:3 * QT4],
                                      c_dram[:, QT4:2 * QT4])
                    nc.gpsimd.tensor_copy(ct[:, QT4:2 * QT4], ct[:, 0:QT4])
                    nc.gpsimd.tensor_copy(ct[:, 3 * QT4:4 * QT4],
                                          ct[:, 2 * QT4:3 * QT4])
                    first = False
                nc.sync.dma_start(qt[:], qt_dram[b, h])
                if h == 0:
                    nc.sync.dma_start(v_bf[:], vt_dram[b])

                # K side: kr_j = ct_j*kt_j + st_j*kt_{j^8}
                t1 = work_pool.tile([P, W], mybir.dt.bfloat16, tag="t1")
                t2 = work_pool.tile([P, W], mybir.dt.bfloat16, tag="t2")
                # separate tiles for the PE/ScalarE-written and DVE-written
                # halves of kr -> no cross-engine writes into one tile
                krA = work_pool.tile([P, KA], mybir.dt.bfloat16, tag="krA")
                krB = work_pool.tile([P, W - KA], mybir.dt.bfloat16, tag="krB")
                # all elementwise on DVE: GPSIMD compute steals the shared
                # SBUF port and slows concurrent DVE ops ~4.5x (measured)
                nc.vector.tensor_mul(t2[:, 0:HW], kt[:, HW:W], st[:, 0:HW])
                nc.vector.tensor_mul(t2[:, HW:W], kt[:, 0:HW], st[:, HW:W])
                nc.vector.tensor_mul(t1[:], kt[:], ct[:])
                # kr cols 0:KA summed on PE (identity accumulate), rest on DVE
                add_ps = ps_add.tile([P, KA], mybir.dt.float32, tag="addps")
                for c3 in range(KA // 512):
                    sl = slice(c3 * 512, (c3 + 1) * 512)
                    nc.tensor.matmul(add_ps[:, sl], ident[:], t1[:, sl],
                                     start=True, stop=False)
                    nc.tensor.matmul(add_ps[:, sl], ident[:], t2[:, sl],
                                     start=False, stop=True)
                nc.scalar.copy(krA[:], add_ps[:])
                nc.vector.tensor_add(krB[:], t1[:, KA:W], t2[:, KA:W])

                # Q side (add folded into mm1): A = ct*qt, B = st_swap*qt
                a_t = work_pool.tile([P, W], mybir.dt.bfloat16, tag="at")
                b_t = work_pool.tile([P, W], mybir.dt.bfloat16, tag="bt")
                # B upper half first: mm1's j=0..7 matmuls consume B_{j^8}
                nc.vector.tensor_mul(b_t[:, HW:W], qt[:, HW:W], st[:, 0:HW])
                nc.vector.tensor_mul(a_t[:], qt[:], ct[:])
                nc.vector.tensor_mul(b_t[:, 0:HW], qt[:, 0:HW], st[:, HW:W])

                # mm1: scoresT[s,t] accumulated over 16 chunks x 2 terms.
                # lhsT = kr chunk j (s-slice), rhs = A_j then B_{j^8}:
                # consecutive matmuls share the stationary operand.
                sc_ps01 = ps_sc.tile([P, TCH * T], mybir.dt.float32, tag="scps")
                sc_ps = [sc_ps01[:, 0:T], sc_ps01[:, T:2 * T]]
                for sch in range(TCH):
                    for j in range(NCH):
                        sj = j ^ (NCH // 2)
                        off = j * T + sch * P
                        if off < KA:
                            lhsT = krA[:, off: off + P]
                        else:
                            lhsT = krB[:, off - KA: off - KA + P]
                        nc.tensor.matmul(sc_ps[sch], lhsT,
                                         a_t[:, j * T:(j + 1) * T],
                                         start=(j == 0), stop=False)
                        nc.tensor.matmul(sc_ps[sch], lhsT,
                                         b_t[:, sj * T:(sj + 1) * T],
                                         start=False, stop=(j == NCH - 1))
                sc_sb = sc_pool.tile([P, TCH * T], mybir.dt.bfloat16, tag="scsb")
                nc.scalar.copy(sc_sb[:], sc_ps01[:])

                # mm2: out[t,d] = sum_s scoresT[s,t] @ V[s,d]
                o_ps = ps_out.tile([P, TCH * D], mybir.dt.float32, tag="ops")
                for tch in range(TCH):
                    for sch in range(TCH):
                        lhsT = sc_sb[:, sch * T + tch * P: sch * T + tch * P + P]
                        rhs = v_bf[:, sch * D:(sch + 1) * D]
                        nc.tensor.matmul(o_ps[:, tch * D:(tch + 1) * D],
                                         lhsT, rhs,
                                         start=(sch == 0), stop=(sch == TCH - 1))
                o_sb = out_pool.tile([P, TCH * D], mybir.dt.bfloat16, tag="osb")
                nc.scalar.copy(o_sb[:], o_ps[:])
                nc.scalar.dma_start(o_dram[b, h], o_sb[:])
    return nc


_NC_CACHE = None


def kernel(Q, K, V):
    global _NC_CACHE, LAST_RESULT
    Q = np.asarray(Q, dtype=np.float32)
    K = np.asarray(K, dtype=np.float32)
    V = np.asarray(V, dtype=np.float32)
    assert Q.shape == (B, H, T, N) and K.shape == (B, H, T, N) and V.shape == (B, 1, T, D)

    if _NC_CACHE is None:
        _NC_CACHE = _build_nc()
        _NC_CACHE.compile()
    nc = _NC_CACHE

    in_maps = _prep_inputs(Q, K, V)

    trace = bool(os.environ.get("BASS_KERNEL_TRACE"))
    res = run_bass_kernel_spmd(nc, in_maps, list(range(NCORES)), trace=trace,
                               trace_cores=[0] if trace else None)
    LAST_RESULT = res
    return _unpack_out([res.results[c]["O"] for c in range(NCORES)])


# revision 40
# speedup vs baseline: 1.0212x; 1.0212x over previous
"""Trainium2 Bass kernel for nn_BidirectionalAttention (B=16,H=4,T=256,N=2048,D=256).

Math (reference):
    Qr = rope2d(Q), Kr = rope2d(K)              # elementwise, per (t, n) angle
    scores = Qr @ Kr^T / sqrt(N)                # (B,H,T,T), no softmax
    out    = scores @ V                         # V (B,1,T,D) broadcasts over H

Strategy (B sharded 2-per-core across 8 cores):
  * Host-side input prep: features are permuted to evens-first order
    (n' = [0,2,..,2046,1,3,..,2047]); the contraction over n is
    permutation-invariant so scores are unchanged.  Under this order the
    RoPE pair-swap becomes a +-1024 BLOCK swap, which in the on-chip
    chunked layout is pure tile indexing (chunk j <-> j^8).
  * Q and K are uploaded bf16 PRE-TRANSPOSED as [n, t] chunk tiles
    ([128, NCH*T] per (b,h)), so the kernel needs NO on-device
    transposes at all (the baseline spent most of its PE time on 128
    transpose matmuls per (b,h)).  bf16 upload also halves HBM traffic.
  * RoPE in transposed space:  QrT_j = cT_j*QT_j + sT_j*QT_{j^8}.
    K side is materialized with 2 muls + 1 add (DVE + GPSIMD).
    Q side's add is FOLDED into mm1's PSUM accumulation:
        scoresT = sum_j  KrT_j^T @ A_j  +  KrT_j^T @ B_{j^8}
    with A = cT.QT, B = sTswap.QT  (2 muls, no add); each lhsT serves
    two consecutive matmuls (one weight load per chunk).
  * mm2: out[t,d] = sum_s scoresT[s,t] @ V[s,d], V uploaded bf16 in
    [s, d] chunk layout (no transpose needed anywhere).
  * 1/sqrt(N) folded into the tables (N^-1/4 on both Q and K sides).
"""

import math
import os
import numpy as np
import ml_dtypes
from contextlib import ExitStack

import concourse.bass as bass
import concourse.bacc as bacc_mod
import concourse.tile as tile
import concourse.mybir as mybir
from concourse.bass_utils import run_bass_kernel_spmd
from concourse.masks import make_identity

bf16 = ml_dtypes.bfloat16

# problem shapes (hardcoded per contract)
B, H, T, N, D = 16, 4, 256, 2048, 256
GRID = 16
THETA = 10000.0
NCORES = 8
BS = B // NCORES          # batches per core
P = 128
NCH = N // P              # 16 feature chunks
TCH = T // P              # 2 token chunks
HALF = N // 2

LAST_RESULT = None        # BassKernelResults of the most recent run (for test.py)


def _tile_pack(x):
    """[T, N]-indexed table -> SBUF tile layout [128, NCH*T]:
    tile[p, j*T + t] = x[t, j*128 + p]."""
    return np.ascontiguousarray(
        x.T.reshape(NCH, P, T).transpose(1, 0, 2).reshape(P, NCH * T))


def _rope_tables():
    """Tables in permuted (evens-first) transposed tile layout, bf16.

    ct[p, j*T+t] = alpha * cos(ang[t, perm[j*128+p]])
    st[p, j*T+t] = alpha * stilde[t, j*128+p]  where
       stilde[t, m <1024] = -sin(ang[t, 2m])      (coeff of Q'[m+1024])
       stilde[t, m>=1024] = +sin(ang[t, 2(m-1024)+1])  (coeff of Q'[m-1024])
    """
    inv_freq = (1.0 / THETA ** (np.arange(0, HALF, 2, dtype=np.float32)
                                / np.float32(HALF))).astype(np.float32)
    pos = np.arange(GRID, dtype=np.float32)
    ph = pos[:, None] * inv_freq[None, :]                      # (16, 512)
    ph_h = np.broadcast_to(ph[:, None, :], (GRID, GRID, HALF // 2))
    ph_w = np.broadcast_to(ph[None, :, :], (GRID, GRID, HALF // 2))
    phases = np.concatenate([ph_h, ph_w, ph_h, ph_w], axis=-1).reshape(T, N)
    ang = np.mod(phases, np.float32(1.0)) * np.float32(2.0 * math.pi)
    c = np.cos(ang)
    s = np.sin(ang)
    alpha = np.float32(1.0 / math.sqrt(math.sqrt(N)))
    perm = np.concatenate([np.arange(0, N, 2), np.arange(1, N, 2)])
    cp = c[:, perm] * alpha
    st = np.empty_like(s)
    st[:, :HALF] = -s[:, 0::2]
    st[:, HALF:] = s[:, 1::2]
    st *= alpha
    return _tile_pack(cp).astype(bf16), _tile_pack(st).astype(bf16)


def _prep_inputs(Q, K, V):
    """Full f32 inputs -> per-core upload dicts (bf16, permuted, transposed,
    chunk-tiled)."""
    perm = np.concatenate([np.arange(0, N, 2), np.arange(1, N, 2)])
    ct, st = _rope_tables()

    def pack_qk(x):
        xp = x[:, :, :, perm].astype(bf16)                   # [B,H,T,N]
        xt = xp.transpose(0, 1, 3, 2)                        # [B,H,N,T]
        return np.ascontiguousarray(
            xt.reshape(B, H, NCH, P, T).transpose(0, 1, 3, 2, 4)
              .reshape(B, H, P, NCH * T))

    qt = pack_qk(Q)
    kt = pack_qk(K)
    vt = np.ascontiguousarray(
        V[:, 0].astype(bf16).reshape(B, TCH, P, D).transpose(0, 2, 1, 3)
         .reshape(B, P, TCH * D))

    in_maps = []
    for c in range(NCORES):
        sl = slice(c * BS, (c + 1) * BS)
        in_maps.append({"QT": qt[sl], "KT": kt[sl], "VT": vt[sl],
                        "CT": ct, "ST": st})
    return in_maps


def _unpack_out(per_core):
    """[BS,H,128,TCH*D] bf16 per core -> [B,H,T,D] f32."""
    o = np.concatenate([np.asarray(p) for p in per_core], axis=0)
    return np.ascontiguousarray(
        o.astype(np.float32).reshape(B, H, P, TCH, D).transpose(0, 1, 3, 2, 4)
         .reshape(B, H, T, D))


def _build_nc():
    nc = bacc_mod.Bacc("TRN2", target_bir_lowering=False, debug=False)

    qt_dram = nc.dram_tensor("QT", [BS, H, P, NCH * T], mybir.dt.bfloat16,
                             kind="ExternalInput").ap()
    kt_dram = nc.dram_tensor("KT", [BS, H, P, NCH * T], mybir.dt.bfloat16,
                             kind="ExternalInput").ap()
    vt_dram = nc.dram_tensor("VT", [BS, P, TCH * D], mybir.dt.bfloat16,
                             kind="ExternalInput").ap()
    c_dram = nc.dram_tensor("CT", [P, NCH * T], mybir.dt.bfloat16,
                            kind="ExternalInput").ap()
    s_dram = nc.dram_tensor("ST", [P, NCH * T], mybir.dt.bfloat16,
                            kind="ExternalInput").ap()
    o_dram = nc.dram_tensor("O", [BS, H, P, TCH * D], mybir.dt.bfloat16,
                            kind="ExternalOutput").ap()

    W = NCH * T          # 4096
    HW = W // 2          # 2048 (chunk-block half: chunks 0-7 | 8-15)

    with tile.TileContext(nc) as tc, ExitStack() as ctx:
        const_pool = ctx.enter_context(tc.tile_pool(name="const", bufs=1))
        qk_pool = ctx.enter_context(tc.tile_pool(name="qk", bufs=2))
        v_pool = ctx.enter_context(tc.tile_pool(name="vp", bufs=2))
        work_pool = ctx.enter_context(tc.tile_pool(name="work", bufs=2))
        sc_pool = ctx.enter_context(tc.tile_pool(name="scp", bufs=2))
        out_pool = ctx.enter_context(tc.tile_pool(name="outp", bufs=2))
        ps_sc = ctx.enter_context(tc.tile_pool(name="pssc", bufs=2, space="PSUM"))
        ps_out = ctx.enter_context(tc.tile_pool(name="psout", bufs=2, space="PSUM"))
        ps_add = ctx.enter_context(tc.tile_pool(name="psadd", bufs=1, space="PSUM"))

        # PE-assisted add: kr chunks 0-5 are summed on the tensor engine
        # (identity-matmul accumulate, exact f32) to offload the DVE
        KA = 6 * T           # 1536 columns of kr via PE, rest via DVE
        ident = const_pool.tile([P, P], mybir.dt.bfloat16)
        make_identity(nc, ident[:])
        ct = const_pool.tile([P, W], mybir.dt.bfloat16)
        st = const_pool.tile([P, W], mybir.dt.bfloat16)
        # ALL loads ride the SP ring in priority order (per-ring transfers
        # complete in issue order; spreading across rings just makes the
        # SDMA round-robin delay the critical ones). Outputs ride the
        # Activation ring. st first (t2 needs kt+st), ct after kt0.
        nc.sync.dma_start(st[:], s_dram)

        first = True
        for b in range(BS):
            v_bf = v_pool.tile([P, TCH * D], mybir.dt.bfloat16, tag="vbf")
            for h in range(H):
                qt = qk_pool.tile([P, W], mybir.dt.bfloat16, tag="qt")
                kt = qk_pool.tile([P, W], mybir.dt.bfloat16, tag="kt")
                nc.sync.dma_start(kt[:], kt_dram[b, h])
                if first:
                    # ct lands after kt0 (needed by t1, the 3rd DVE op)
                    nc.sync.dma_start(ct[:], c_dram)
                    first = False
                nc.sync.dma_start(qt[:], qt_dram[b, h])
                if h == 0:
                    nc.sync.dma_start(v_bf[:], vt_dram[b])

                # K side: kr_j = ct_j*kt_j + st_j*kt_{j^8}
                t1 = work_pool.tile([P, W], mybir.dt.bfloat16, tag="t1")
                t2 = work_pool.tile([P, W], mybir.dt.bfloat16, tag="t2")
                # separate tiles for the PE/ScalarE-written and DVE-written
                # halves of kr -> no cross-engine writes into one tile
                krA = work_pool.tile([P, KA], mybir.dt.bfloat16, tag="krA")
                krB = work_pool.tile([P, W - KA], mybir.dt.bfloat16, tag="krB")
                # all elementwise on DVE: GPSIMD compute steals the shared
                # SBUF port and slows concurrent DVE ops ~4.5x (measured)
                nc.vector.tensor_mul(t2[:, 0:HW], kt[:, HW:W], st[:, 0:HW])
                nc.vector.tensor_mul(t2[:, HW:W], kt[:, 0:HW], st[:, HW:W])
                nc.vector.tensor_mul(t1[:], kt[:], ct[:])
                # kr cols 0:KA summed on PE (identity accumulate), rest on DVE
                add_ps = ps_add.tile([P, KA], mybir.dt.float32, tag="addps")
                for c3 in range(KA // 512):
                    sl = slice(c3 * 512, (c3 + 1) * 512)
                    nc.tensor.matmul(add_ps[:, sl], ident[:], t1[:, sl],
                                     start=True, stop=False)
                    nc.tensor.matmul(add_ps[:, sl], ident[:], t2[:, sl],
                                     start=False, stop=True)
                nc.scalar.copy(krA[:], add_ps[:])
                nc.vector.tensor_add(krB[:], t1[:, KA:W], t2[:, KA:W])

                # Q side (add folded into mm1): A = ct*qt, B = st_swap*qt
                a_t = work_pool.tile([P, W], mybir.dt.bfloat16, tag="at")
                b_t = work_pool.tile([P, W], mybir.dt.bfloat16, tag="bt")
                # halves ordered so mm1's j=0..7 operands (A lower half,
                # B upper half) land first -> mm1 overlaps the remaining
                # DVE work (shortens the last-iteration drain)
                nc.vector.tensor_mul(b_t[:, HW:W], qt[:, HW:W], st[:, 0:HW])
                nc.vector.tensor_mul(a_t[:, 0:HW], qt[:, 0:HW], ct[:, 0:HW])
                nc.vector.tensor_mul(a_t[:, HW:W], qt[:, HW:W], ct[:, HW:W])
                nc.vector.tensor_mul(b_t[:, 0:HW], qt[:, 0:HW], st[:, HW:W])

                # mm1: scoresT[s,t] accumulated over 16 chunks x 2 terms.
                # lhsT = kr chunk j (s-slice), rhs = A_j then B_{j^8}:
                # consecutive matmuls share the stationary operand.
                sc_ps01 = ps_sc.tile([P, TCH * T], mybir.dt.float32, tag="scps")
                sc_ps = [sc_ps01[:, 0:T], sc_ps01[:, T:2 * T]]
                for sch in range(TCH):
                    for j in range(NCH):
                        sj = j ^ (NCH // 2)
                        off = j * T + sch * P
                        if off < KA:
                            lhsT = krA[:, off: off + P]
                        else:
                            lhsT = krB[:, off - KA: off - KA + P]
                        nc.tensor.matmul(sc_ps[sch], lhsT,
                                         a_t[:, j * T:(j + 1) * T],
                                         start=(j == 0), stop=False)
                        nc.tensor.matmul(sc_ps[sch], lhsT,
                                         b_t[:, sj * T:(sj + 1) * T],
                                         start=False, stop=(j == NCH - 1))
                sc_sb = sc_pool.tile([P, TCH * T], mybir.dt.bfloat16, tag="scsb")
                for sch in range(TCH):
                    nc.scalar.copy(sc_sb[:, sch * T:(sch + 1) * T], sc_ps[sch])

                # mm2: out[t,d] = sum_s scoresT[s,t] @ V[s,d]
                o_ps = ps_out.tile([P, TCH * D], mybir.dt.float32, tag="ops")
                for tch in range(TCH):
                    for sch in range(TCH):
                        lhsT = sc_sb[:, sch * T + tch * P: sch * T + tch * P + P]
                        rhs = v_bf[:, sch * D:(sch + 1) * D]
                        nc.tensor.matmul(o_ps[:, tch * D:(tch + 1) * D],
                                         lhsT, rhs,
                                         start=(sch == 0), stop=(sch == TCH - 1))
                o_sb = out_pool.tile([P, TCH * D], mybir.dt.bfloat16, tag="osb")
                nc.scalar.copy(o_sb[:], o_ps[:])
                nc.scalar.dma_start(o_dram[b, h], o_sb[:])
    return nc


_NC_CACHE = None


def kernel(Q, K, V):
    global _NC_CACHE, LAST_RESULT
    Q = np.asarray(Q, dtype=np.float32)
    K = np.asarray(K, dtype=np.float32)
    V = np.asarray(V, dtype=np.float32)
    assert Q.shape == (B, H, T, N) and K.shape == (B, H, T, N) and V.shape == (B, 1, T, D)

    if _NC_CACHE is None:
        _NC_CACHE = _build_nc()
        _NC_CACHE.compile()
    nc = _NC_CACHE

    in_maps = _prep_inputs(Q, K, V)

    trace = bool(os.environ.get("BASS_KERNEL_TRACE"))
    res = run_bass_kernel_spmd(nc, in_maps, list(range(NCORES)), trace=trace,
                               trace_cores=[0] if trace else None)
    LAST_RESULT = res
    return _unpack_out([res.results[c]["O"] for c in range(NCORES)])


# revision 43
# speedup vs baseline: 1.0851x; 1.0626x over previous
"""Trainium2 Bass kernel for nn_BidirectionalAttention (B=16,H=4,T=256,N=2048,D=256).

Math (reference):
    Qr = rope2d(Q), Kr = rope2d(K)              # elementwise, per (t, n) angle
    scores = Qr @ Kr^T / sqrt(N)                # (B,H,T,T), no softmax
    out    = scores @ V                         # V (B,1,T,D) broadcasts over H

Strategy (B sharded 2-per-core across 8 cores):
  * Host-side input prep: features are permuted to evens-first order
    (n' = [0,2,..,2046,1,3,..,2047]); the contraction over n is
    permutation-invariant so scores are unchanged.  Under this order the
    RoPE pair-swap becomes a +-1024 BLOCK swap, which in the on-chip
    chunked layout is pure tile indexing (chunk j <-> j^8).
  * Q and K are uploaded bf16 PRE-TRANSPOSED as [n, t] chunk tiles
    ([128, NCH*T] per (b,h)), so the kernel needs NO on-device
    transposes at all (the baseline spent most of its PE time on 128
    transpose matmuls per (b,h)).  bf16 upload also halves HBM traffic.
  * RoPE in transposed space:  QrT_j = cT_j*QT_j + sT_j*QT_{j^8}.
    K side is materialized with 2 DVE muls + an add that is split between
    the tensor engine (identity-matmul PSUM accumulate, chunks 0-5) and
    the DVE (rest).  GPSIMD is kept idle: its SBUF-port sharing slows
    concurrent DVE ops ~4.5x (measured).
    Q side's add is FOLDED into mm1's PSUM accumulation:
        scoresT = sum_j  KrT_j^T @ A_j  +  KrT_j^T @ B_{j^8}
    with A = cT.QT, B = sTswap.QT  (2 muls, no add); each lhsT serves
    two consecutive matmuls (one weight load per chunk).
  * mm2: out[t,d] = sum_s scoresT[s,t] @ V[s,d], V uploaded bf16 in
    [s, d] chunk layout (no transpose needed anywhere).
  * 1/sqrt(N) folded into the tables (N^-1/4 on both Q and K sides).
"""

import math
import os
import numpy as np
import ml_dtypes
from contextlib import ExitStack

import concourse.bass as bass
import concourse.bacc as bacc_mod
import concourse.tile as tile
import concourse.mybir as mybir
from concourse.bass_utils import run_bass_kernel_spmd
from concourse.masks import make_identity

bf16 = ml_dtypes.bfloat16

# problem shapes (hardcoded per contract)
B, H, T, N, D = 16, 4, 256, 2048, 256
GRID = 16
THETA = 10000.0
NCORES = 8
BS = B // NCORES          # batches per core
P = 128
NCH = N // P              # 16 feature chunks
TCH = T // P              # 2 token chunks
HALF = N // 2

LAST_RESULT = None        # BassKernelResults of the most recent run (for test.py)


def _tile_pack(x):
    """[T, N]-indexed table -> SBUF tile layout [128, NCH*T]:
    tile[p, j*T + t] = x[t, j*128 + p]."""
    return np.ascontiguousarray(
        x.T.reshape(NCH, P, T).transpose(1, 0, 2).reshape(P, NCH * T))


def _rope_tables():
    """Tables in permuted (evens-first) transposed tile layout, bf16.

    ct[p, j*T+t] = alpha * cos(ang[t, perm[j*128+p]])
    st[p, j*T+t] = alpha * stilde[t, j*128+p]  where
       stilde[t, m <1024] = -sin(ang[t, 2m])      (coeff of Q'[m+1024])
       stilde[t, m>=1024] = +sin(ang[t, 2(m-1024)+1])  (coeff of Q'[m-1024])
    """
    inv_freq = (1.0 / THETA ** (np.arange(0, HALF, 2, dtype=np.float32)
                                / np.float32(HALF))).astype(np.float32)
    pos = np.arange(GRID, dtype=np.float32)
    ph = pos[:, None] * inv_freq[None, :]                      # (16, 512)
    ph_h = np.broadcast_to(ph[:, None, :], (GRID, GRID, HALF // 2))
    ph_w = np.broadcast_to(ph[None, :, :], (GRID, GRID, HALF // 2))
    phases = np.concatenate([ph_h, ph_w, ph_h, ph_w], axis=-1).reshape(T, N)
    ang = np.mod(phases, np.float32(1.0)) * np.float32(2.0 * math.pi)
    c = np.cos(ang)
    s = np.sin(ang)
    alpha = np.float32(1.0 / math.sqrt(math.sqrt(N)))
    perm = np.concatenate([np.arange(0, N, 2), np.arange(1, N, 2)])
    cp = c[:, perm] * alpha
    st = np.empty_like(s)
    st[:, :HALF] = -s[:, 0::2]
    st[:, HALF:] = s[:, 1::2]
    st *= alpha
    return _tile_pack(cp).astype(bf16), _tile_pack(st).astype(bf16)


def _prep_inputs(Q, K, V):
    """Full f32 inputs -> per-core upload dicts (bf16, permuted, transposed,
    chunk-tiled)."""
    perm = np.concatenate([np.arange(0, N, 2), np.arange(1, N, 2)])
    ct, st = _rope_tables()

    def pack_qk(x):
        xp = x[:, :, :, perm].astype(bf16)                   # [B,H,T,N]
        xt = xp.transpose(0, 1, 3, 2)                        # [B,H,N,T]
        return np.ascontiguousarray(
            xt.reshape(B, H, NCH, P, T).transpose(0, 1, 3, 2, 4)
              .reshape(B, H, P, NCH * T))

    qt = pack_qk(Q)
    kt = pack_qk(K)
    vt = np.ascontiguousarray(
        V[:, 0].astype(bf16).reshape(B, TCH, P, D).transpose(0, 2, 1, 3)
         .reshape(B, P, TCH * D))

    in_maps = []
    for c in range(NCORES):
        sl = slice(c * BS, (c + 1) * BS)
        in_maps.append({"QT": qt[sl], "KT": kt[sl], "VT": vt[sl],
                        "CT": ct, "ST": st})
    return in_maps


def _unpack_out(per_core):
    """[BS,H,128,TCH*D] bf16 per core -> [B,H,T,D] f32."""
    o = np.concatenate([np.asarray(p) for p in per_core], axis=0)
    return np.ascontiguousarray(
        o.astype(np.float32).reshape(B, H, P, TCH, D).transpose(0, 1, 3, 2, 4)
         .reshape(B, H, T, D))


def _build_nc():
    nc = bacc_mod.Bacc("TRN2", target_bir_lowering=False, debug=False)

    qt_dram = nc.dram_tensor("QT", [BS, H, P, NCH * T], mybir.dt.bfloat16,
                             kind="ExternalInput").ap()
    kt_dram = nc.dram_tensor("KT", [BS, H, P, NCH * T], mybir.dt.bfloat16,
                             kind="ExternalInput").ap()
    vt_dram = nc.dram_tensor("VT", [BS, P, TCH * D], mybir.dt.bfloat16,
                             kind="ExternalInput").ap()
    c_dram = nc.dram_tensor("CT", [P, NCH * T], mybir.dt.bfloat16,
                            kind="ExternalInput").ap()
    s_dram = nc.dram_tensor("ST", [P, NCH * T], mybir.dt.bfloat16,
                            kind="ExternalInput").ap()
    o_dram = nc.dram_tensor("O", [BS, H, P, TCH * D], mybir.dt.bfloat16,
                            kind="ExternalOutput").ap()

    W = NCH * T          # 4096
    HW = W // 2          # 2048 (chunk-block half: chunks 0-7 | 8-15)

    with tile.TileContext(nc) as tc, ExitStack() as ctx:
        const_pool = ctx.enter_context(tc.tile_pool(name="const", bufs=1))
        qk_pool = ctx.enter_context(tc.tile_pool(name="qk", bufs=2))
        v_pool = ctx.enter_context(tc.tile_pool(name="vp", bufs=2))
        work_pool = ctx.enter_context(tc.tile_pool(name="work", bufs=2))
        sc_pool = ctx.enter_context(tc.tile_pool(name="scp", bufs=2))
        out_pool = ctx.enter_context(tc.tile_pool(name="outp", bufs=2))
        ps_sc = ctx.enter_context(tc.tile_pool(name="pssc", bufs=2, space="PSUM"))
        ps_out = ctx.enter_context(tc.tile_pool(name="psout", bufs=2, space="PSUM"))
        ps_add = ctx.enter_context(tc.tile_pool(name="psadd", bufs=1, space="PSUM"))

        # PE-assisted add: kr chunks 0-5 are summed on the tensor engine
        # (identity-matmul accumulate, exact f32) to offload the DVE
        KA = 6 * T           # 1536 columns of kr via PE, rest via DVE
        ident = const_pool.tile([P, P], mybir.dt.bfloat16)
        make_identity(nc, ident[:])
        ct = const_pool.tile([P, W], mybir.dt.bfloat16)
        st = const_pool.tile([P, W], mybir.dt.bfloat16)
        # ALL loads ride the SP ring in priority order (per-ring transfers
        # complete in issue order; spreading across rings just makes the
        # SDMA round-robin delay the critical ones). Outputs ride the
        # Activation ring. st first (t2 needs kt+st), ct after kt0.
        nc.sync.dma_start(st[:], s_dram)

        first = True
        for b in range(BS):
            v_bf = v_pool.tile([P, TCH * D], mybir.dt.bfloat16, tag="vbf")
            for h in range(H):
                qt = qk_pool.tile([P, W], mybir.dt.bfloat16, tag="qt")
                kt = qk_pool.tile([P, W], mybir.dt.bfloat16, tag="kt")
                nc.sync.dma_start(kt[:], kt_dram[b, h])
                if first:
                    # ct lands after kt0 (needed by t1, the 3rd DVE op)
                    nc.sync.dma_start(ct[:], c_dram)
                    first = False
                nc.sync.dma_start(qt[:], qt_dram[b, h])
                if h == 0:
                    nc.sync.dma_start(v_bf[:], vt_dram[b])

                # K side: kr_j = ct_j*kt_j + st_j*kt_{j^8}
                t1 = work_pool.tile([P, W], mybir.dt.bfloat16, tag="t1")
                t2 = work_pool.tile([P, W], mybir.dt.bfloat16, tag="t2")
                # separate tiles for the PE/ScalarE-written and DVE-written
                # halves of kr -> no cross-engine writes into one tile
                krA = work_pool.tile([P, KA], mybir.dt.bfloat16, tag="krA")
                krB = work_pool.tile([P, W - KA], mybir.dt.bfloat16, tag="krB")
                # all elementwise on DVE: GPSIMD compute steals the shared
                # SBUF port and slows concurrent DVE ops ~4.5x (measured)
                nc.vector.tensor_mul(t2[:, 0:HW], kt[:, HW:W], st[:, 0:HW])
                nc.vector.tensor_mul(t2[:, HW:W], kt[:, 0:HW], st[:, HW:W])
                nc.vector.tensor_mul(t1[:], kt[:], ct[:])
                # kr cols 0:KA summed on PE (identity accumulate), rest on DVE
                add_ps = ps_add.tile([P, KA], mybir.dt.float32, tag="addps")
                for c3 in range(KA // 512):
                    sl = slice(c3 * 512, (c3 + 1) * 512)
                    nc.tensor.matmul(add_ps[:, sl], ident[:], t1[:, sl],
                                     start=True, stop=False)
                    nc.tensor.matmul(add_ps[:, sl], ident[:], t2[:, sl],
                                     start=False, stop=True)
                nc.scalar.copy(krA[:], add_ps[:])
                nc.vector.tensor_add(krB[:], t1[:, KA:W], t2[:, KA:W])

                # Q side (add folded into mm1): A = ct*qt, B = st_swap*qt
                a_t = work_pool.tile([P, W], mybir.dt.bfloat16, tag="at")
                b_t = work_pool.tile([P, W], mybir.dt.bfloat16, tag="bt")
                # B upper half first: mm1's j=0..7 matmuls consume B_{j^8}
                nc.vector.tensor_mul(b_t[:, HW:W], qt[:, HW:W], st[:, 0:HW])
                nc.vector.tensor_mul(a_t[:], qt[:], ct[:])
                nc.vector.tensor_mul(b_t[:, 0:HW], qt[:, 0:HW], st[:, HW:W])

                # mm1: scoresT[s,t] accumulated over 16 chunks x 2 terms.
                # lhsT = kr chunk j (s-slice), rhs = A_j then B_{j^8}:
                # consecutive matmuls share the stationary operand.
                sc_ps01 = ps_sc.tile([P, TCH * T], mybir.dt.float32, tag="scps")
                sc_ps = [sc_ps01[:, 0:T], sc_ps01[:, T:2 * T]]
                for sch in range(TCH):
                    for j in range(NCH):
                        sj = j ^ (NCH // 2)
                        off = j * T + sch * P
                        if off < KA:
                            lhsT = krA[:, off: off + P]
                        else:
                            lhsT = krB[:, off - KA: off - KA + P]
                        nc.tensor.matmul(sc_ps[sch], lhsT,
                                         a_t[:, j * T:(j + 1) * T],
                                         start=(j == 0), stop=False)
                        nc.tensor.matmul(sc_ps[sch], lhsT,
                                         b_t[:, sj * T:(sj + 1) * T],
                                         start=False, stop=(j == NCH - 1))
                sc_sb = sc_pool.tile([P, TCH * T], mybir.dt.bfloat16, tag="scsb")
                nc.scalar.copy(sc_sb[:], sc_ps01[:])

                # mm2: out[t,d] = sum_s scoresT[s,t] @ V[s,d]
                o_ps = ps_out.tile([P, TCH * D], mybir.dt.float32, tag="ops")
                for tch in range(TCH):
                    for sch in range(TCH):
                        lhsT = sc_sb[:, sch * T + tch * P: sch * T + tch * P + P]
                        rhs = v_bf[:, sch * D:(sch + 1) * D]
                        nc.tensor.matmul(o_ps[:, tch * D:(tch + 1) * D],
                                         lhsT, rhs,
                                         start=(sch == 0), stop=(sch == TCH - 1))
                o_sb = out_pool.tile([P, TCH * D], mybir.dt.bfloat16, tag="osb")
                nc.scalar.copy(o_sb[:], o_ps[:])
                nc.scalar.dma_start(o_dram[b, h], o_sb[:])
    return nc


_NC_CACHE = None


def kernel(Q, K, V):
    global _NC_CACHE, LAST_RESULT
    Q = np.asarray(Q, dtype=np.float32)
    K = np.asarray(K, dtype=np.float32)
    V = np.asarray(V, dtype=np.float32)
    assert Q.shape == (B, H, T, N) and K.shape == (B, H, T, N) and V.shape == (B, 1, T, D)

    if _NC_CACHE is None:
        _NC_CACHE = _build_nc()
        _NC_CACHE.compile()
    nc = _NC_CACHE

    in_maps = _prep_inputs(Q, K, V)

    trace = bool(os.environ.get("BASS_KERNEL_TRACE"))
    res = run_bass_kernel_spmd(nc, in_maps, list(range(NCORES)), trace=trace,
                               trace_cores=[0] if trace else None)
    LAST_RESULT = res
    return _unpack_out([res.results[c]["O"] for c in range(NCORES)])


# revision 44
# speedup vs baseline: 1.0982x; 1.0121x over previous
"""Trainium2 Bass kernel for nn_BidirectionalAttention (B=16,H=4,T=256,N=2048,D=256).

Math (reference):
    Qr = rope2d(Q), Kr = rope2d(K)              # elementwise, per (t, n) angle
    scores = Qr @ Kr^T / sqrt(N)                # (B,H,T,T), no softmax
    out    = scores @ V                         # V (B,1,T,D) broadcasts over H

Strategy (B sharded 2-per-core across 8 cores):
  * Host-side input prep: features are permuted to evens-first order
    (n' = [0,2,..,2046,1,3,..,2047]); the contraction over n is
    permutation-invariant so scores are unchanged.  Under this order the
    RoPE pair-swap becomes a +-1024 BLOCK swap, which in the on-chip
    chunked layout is pure tile indexing (chunk j <-> j^8).
  * Q and K are uploaded bf16 PRE-TRANSPOSED as [n, t] chunk tiles
    ([128, NCH*T] per (b,h)), so the kernel needs NO on-device
    transposes at all (the baseline spent most of its PE time on 128
    transpose matmuls per (b,h)).  bf16 upload also halves HBM traffic.
  * RoPE in transposed space:  QrT_j = cT_j*QT_j + sT_j*QT_{j^8}.
    K side is materialized with 2 DVE muls + an add that is split between
    the tensor engine (identity-matmul PSUM accumulate, chunks 0-5) and
    the DVE (rest).  GPSIMD is kept idle: its SBUF-port sharing slows
    concurrent DVE ops ~4.5x (measured).
    Q side's add is FOLDED into mm1's PSUM accumulation:
        scoresT = sum_j  KrT_j^T @ A_j  +  KrT_j^T @ B_{j^8}
    with A = cT.QT, B = sTswap.QT  (2 muls, no add); each lhsT serves
    two consecutive matmuls (one weight load per chunk).
  * mm2: out[t,d] = sum_s scoresT[s,t] @ V[s,d], V uploaded bf16 in
    [s, d] chunk layout (no transpose needed anywhere).
  * 1/sqrt(N) folded into the tables (N^-1/4 on both Q and K sides).
"""

import math
import os
import numpy as np
import ml_dtypes
from contextlib import ExitStack

import concourse.bass as bass
import concourse.bacc as bacc_mod
import concourse.tile as tile
import concourse.mybir as mybir
from concourse.bass_utils import run_bass_kernel_spmd
from concourse.masks import make_identity

bf16 = ml_dtypes.bfloat16

# problem shapes (hardcoded per contract)
B, H, T, N, D = 16, 4, 256, 2048, 256
GRID = 16
THETA = 10000.0
NCORES = 8
BS = B // NCORES          # batches per core
P = 128
NCH = N // P              # 16 feature chunks
TCH = T // P              # 2 token chunks
HALF = N // 2

LAST_RESULT = None        # BassKernelResults of the most recent run (for test.py)


def _tile_pack(x):
    """[T, N]-indexed table -> SBUF tile layout [128, NCH*T]:
    tile[p, j*T + t] = x[t, j*128 + p]."""
    return np.ascontiguousarray(
        x.T.reshape(NCH, P, T).transpose(1, 0, 2).reshape(P, NCH * T))


def _rope_tables():
    """Tables in permuted (evens-first) transposed tile layout, bf16.

    ct[p, j*T+t] = alpha * cos(ang[t, perm[j*128+p]])
    st[p, j*T+t] = alpha * stilde[t, j*128+p]  where
       stilde[t, m <1024] = -sin(ang[t, 2m])      (coeff of Q'[m+1024])
       stilde[t, m>=1024] = +sin(ang[t, 2(m-1024)+1])  (coeff of Q'[m-1024])
    """
    inv_freq = (1.0 / THETA ** (np.arange(0, HALF, 2, dtype=np.float32)
                                / np.float32(HALF))).astype(np.float32)
    pos = np.arange(GRID, dtype=np.float32)
    ph = pos[:, None] * inv_freq[None, :]                      # (16, 512)
    ph_h = np.broadcast_to(ph[:, None, :], (GRID, GRID, HALF // 2))
    ph_w = np.broadcast_to(ph[None, :, :], (GRID, GRID, HALF // 2))
    phases = np.concatenate([ph_h, ph_w, ph_h, ph_w], axis=-1).reshape(T, N)
    ang = np.mod(phases, np.float32(1.0)) * np.float32(2.0 * math.pi)
    c = np.cos(ang)
    s = np.sin(ang)
    alpha = np.float32(1.0 / math.sqrt(math.sqrt(N)))
    perm = np.concatenate([np.arange(0, N, 2), np.arange(1, N, 2)])
    cp = c[:, perm] * alpha
    st = np.empty_like(s)
    st[:, :HALF] = -s[:, 0::2]
    st[:, HALF:] = s[:, 1::2]
    st *= alpha
    return _tile_pack(cp).astype(bf16), _tile_pack(st).astype(bf16)


def _prep_inputs(Q, K, V):
    """Full f32 inputs -> per-core upload dicts (bf16, permuted, transposed,
    chunk-tiled)."""
    perm = np.concatenate([np.arange(0, N, 2), np.arange(1, N, 2)])
    ct, st = _rope_tables()

    def pack_qk(x):
        xp = x[:, :, :, perm].astype(bf16)                   # [B,H,T,N]
        xt = xp.transpose(0, 1, 3, 2)                        # [B,H,N,T]
        return np.ascontiguousarray(
            xt.reshape(B, H, NCH, P, T).transpose(0, 1, 3, 2, 4)
              .reshape(B, H, P, NCH * T))

    qt = pack_qk(Q)
    kt = pack_qk(K)
    vt = np.ascontiguousarray(
        V[:, 0].astype(bf16).reshape(B, TCH, P, D).transpose(0, 2, 1, 3)
         .reshape(B, P, TCH * D))

    in_maps = []
    for c in range(NCORES):
        sl = slice(c * BS, (c + 1) * BS)
        in_maps.append({"QT": qt[sl], "KT": kt[sl], "VT": vt[sl],
                        "CT": ct, "ST": st})
    return in_maps


def _unpack_out(per_core):
    """[BS,H,128,TCH*D] bf16 per core -> [B,H,T,D] f32."""
    o = np.concatenate([np.asarray(p) for p in per_core], axis=0)
    return np.ascontiguousarray(
        o.astype(np.float32).reshape(B, H, P, TCH, D).transpose(0, 1, 3, 2, 4)
         .reshape(B, H, T, D))


def _build_nc():
    nc = bacc_mod.Bacc("TRN2", target_bir_lowering=False, debug=False)

    qt_dram = nc.dram_tensor("QT", [BS, H, P, NCH * T], mybir.dt.bfloat16,
                             kind="ExternalInput").ap()
    kt_dram = nc.dram_tensor("KT", [BS, H, P, NCH * T], mybir.dt.bfloat16,
                             kind="ExternalInput").ap()
    vt_dram = nc.dram_tensor("VT", [BS, P, TCH * D], mybir.dt.bfloat16,
                             kind="ExternalInput").ap()
    c_dram = nc.dram_tensor("CT", [P, NCH * T], mybir.dt.bfloat16,
                            kind="ExternalInput").ap()
    s_dram = nc.dram_tensor("ST", [P, NCH * T], mybir.dt.bfloat16,
                            kind="ExternalInput").ap()
    o_dram = nc.dram_tensor("O", [BS, H, P, TCH * D], mybir.dt.bfloat16,
                            kind="ExternalOutput").ap()

    W = NCH * T          # 4096
    HW = W // 2          # 2048 (chunk-block half: chunks 0-7 | 8-15)

    with tile.TileContext(nc) as tc, ExitStack() as ctx:
        const_pool = ctx.enter_context(tc.tile_pool(name="const", bufs=1))
        qk_pool = ctx.enter_context(tc.tile_pool(name="qk", bufs=3))
        v_pool = ctx.enter_context(tc.tile_pool(name="vp", bufs=2))
        work_pool = ctx.enter_context(tc.tile_pool(name="work", bufs=2))
        sc_pool = ctx.enter_context(tc.tile_pool(name="scp", bufs=2))
        out_pool = ctx.enter_context(tc.tile_pool(name="outp", bufs=2))
        ps_sc = ctx.enter_context(tc.tile_pool(name="pssc", bufs=2, space="PSUM"))
        ps_out = ctx.enter_context(tc.tile_pool(name="psout", bufs=2, space="PSUM"))
        ps_add = ctx.enter_context(tc.tile_pool(name="psadd", bufs=1, space="PSUM"))

        # PE-assisted add: kr chunks 0-5 are summed on the tensor engine
        # (identity-matmul accumulate, exact f32) to offload the DVE
        KA = 6 * T           # 1536 columns of kr via PE, rest via DVE
        ident = const_pool.tile([P, P], mybir.dt.bfloat16)
        make_identity(nc, ident[:])
        ct = const_pool.tile([P, W], mybir.dt.bfloat16)
        st = const_pool.tile([P, W], mybir.dt.bfloat16)
        # ALL loads ride the SP ring in priority order (per-ring transfers
        # complete in issue order; spreading across rings just makes the
        # SDMA round-robin delay the critical ones). Outputs ride the
        # Activation ring. st first (t2 needs kt+st), ct after kt0.
        nc.sync.dma_start(st[:], s_dram)

        first = True
        for b in range(BS):
            v_bf = v_pool.tile([P, TCH * D], mybir.dt.bfloat16, tag="vbf")
            for h in range(H):
                qt = qk_pool.tile([P, W], mybir.dt.bfloat16, tag="qt")
                kt = qk_pool.tile([P, W], mybir.dt.bfloat16, tag="kt")
                nc.sync.dma_start(kt[:], kt_dram[b, h])
                if first:
                    # ct lands after kt0 (needed by t1, the 3rd DVE op)
                    nc.sync.dma_start(ct[:], c_dram)
                    first = False
                nc.sync.dma_start(qt[:], qt_dram[b, h])
                if h == 0:
                    nc.sync.dma_start(v_bf[:], vt_dram[b])

                # K side: kr_j = ct_j*kt_j + st_j*kt_{j^8}
                t1 = work_pool.tile([P, W], mybir.dt.bfloat16, tag="t1")
                t2 = work_pool.tile([P, W], mybir.dt.bfloat16, tag="t2")
                # separate tiles for the PE/ScalarE-written and DVE-written
                # halves of kr -> no cross-engine writes into one tile
                krA = work_pool.tile([P, KA], mybir.dt.bfloat16, tag="krA")
                krB = work_pool.tile([P, W - KA], mybir.dt.bfloat16, tag="krB")
                # all elementwise on DVE: GPSIMD compute steals the shared
                # SBUF port and slows concurrent DVE ops ~4.5x (measured)
                nc.vector.tensor_mul(t2[:, 0:HW], kt[:, HW:W], st[:, 0:HW])
                nc.vector.tensor_mul(t2[:, HW:W], kt[:, 0:HW], st[:, HW:W])
                nc.vector.tensor_mul(t1[:], kt[:], ct[:])
                # kr cols 0:KA summed on PE (identity accumulate), rest on DVE
                add_ps = ps_add.tile([P, KA], mybir.dt.float32, tag="addps")
                for c3 in range(KA // 512):
                    sl = slice(c3 * 512, (c3 + 1) * 512)
                    nc.tensor.matmul(add_ps[:, sl], ident[:], t1[:, sl],
                                     start=True, stop=False)
                    nc.tensor.matmul(add_ps[:, sl], ident[:], t2[:, sl],
                                     start=False, stop=True)
                nc.scalar.copy(krA[:], add_ps[:])
                nc.vector.tensor_add(krB[:], t1[:, KA:W], t2[:, KA:W])

                # Q side (add folded into mm1): A = ct*qt, B = st_swap*qt
                a_t = work_pool.tile([P, W], mybir.dt.bfloat16, tag="at")
                b_t = work_pool.tile([P, W], mybir.dt.bfloat16, tag="bt")
                # B upper half first: mm1's j=0..7 matmuls consume B_{j^8}
                nc.vector.tensor_mul(b_t[:, HW:W], qt[:, HW:W], st[:, 0:HW])
                nc.vector.tensor_mul(a_t[:], qt[:], ct[:])
                nc.vector.tensor_mul(b_t[:, 0:HW], qt[:, 0:HW], st[:, HW:W])

                # mm1: scoresT[s,t] accumulated over 16 chunks x 2 terms.
                # lhsT = kr chunk j (s-slice), rhs = A_j then B_{j^8}:
                # consecutive matmuls share the stationary operand.
                sc_ps01 = ps_sc.tile([P, TCH * T], mybir.dt.float32, tag="scps")
                sc_ps = [sc_ps01[:, 0:T], sc_ps01[:, T:2 * T]]
                for sch in range(TCH):
                    for j in range(NCH):
                        sj = j ^ (NCH // 2)
                        off = j * T + sch * P
                        if off < KA:
                            lhsT = krA[:, off: off + P]
                        else:
                            lhsT = krB[:, off - KA: off - KA + P]
                        nc.tensor.matmul(sc_ps[sch], lhsT,
                                         a_t[:, j * T:(j + 1) * T],
                                         start=(j == 0), stop=False)
                        nc.tensor.matmul(sc_ps[sch], lhsT,
                                         b_t[:, sj * T:(sj + 1) * T],
                                         start=False, stop=(j == NCH - 1))
                sc_sb = sc_pool.tile([P, TCH * T], mybir.dt.bfloat16, tag="scsb")
                nc.scalar.copy(sc_sb[:], sc_ps01[:])

                # mm2: out[t,d] = sum_s scoresT[s,t] @ V[s,d]
                o_ps = ps_out.tile([P, TCH * D], mybir.dt.float32, tag="ops")
                for tch in range(TCH):
                    for sch in range(TCH):
                        lhsT = sc_sb[:, sch * T + tch * P: sch * T + tch * P + P]
                        rhs = v_bf[:, sch * D:(sch + 1) * D]
                        nc.tensor.matmul(o_ps[:, tch * D:(tch + 1) * D],
                                         lhsT, rhs,
                                         start=(sch == 0), stop=(sch == TCH - 1))
                o_sb = out_pool.tile([P, TCH * D], mybir.dt.bfloat16, tag="osb")
                nc.scalar.copy(o_sb[:], o_ps[:])
                nc.scalar.dma_start(o_dram[b, h], o_sb[:])
    return nc


_NC_CACHE = None


def kernel(Q, K, V):
    global _NC_CACHE, LAST_RESULT
    Q = np.asarray(Q, dtype=np.float32)
    K = np.asarray(K, dtype=np.float32)
    V = np.asarray(V, dtype=np.float32)
    assert Q.shape == (B, H, T, N) and K.shape == (B, H, T, N) and V.shape == (B, 1, T, D)

    if _NC_CACHE is None:
        _NC_CACHE = _build_nc()
        _NC_CACHE.compile()
    nc = _NC_CACHE

    in_maps = _prep_inputs(Q, K, V)

    trace = bool(os.environ.get("BASS_KERNEL_TRACE"))
    res = run_bass_kernel_spmd(nc, in_maps, list(range(NCORES)), trace=trace,
                               trace_cores=[0] if trace else None)
    LAST_RESULT = res
    return _unpack_out([res.results[c]["O"] for c in range(NCORES)])
